# revision 65
# baseline (speedup 1.0000x reference)
"""Trainium2 Bass kernel for nn_EpiSIGNetV5 (sparse_attention).

Strategy: data-parallel over batch B=8 across the 8 NeuronCores (1 batch each).
Per core, the whole network is fused on-chip:
  - delayed signal + combined features built transposed ([33, N] incl. a ones
    row so matmul biases ride along in the contraction)
  - QKV via a fused projection (low@high collapsed on host); q&u / k&v are
    assembled into 16-row head groups by a single matmul per head that both
    projects (W block) and passes through u/v (identity block)
  - attention scores computed TRANSPOSED ([m, n] tiles) with the graph bias
    u@v folded into the same K=16 contraction; exp() runs straight out of
    PSUM on the scalar engine; the softmax denominator comes from a ones
    column appended to V in the PV matmul, so no transposes and no
    partition-dim reductions are ever needed.
  - QK matmuls are 4-way row-group packed (one head per 32-row PE group);
    PV matmuls are 4-way col-group packed (one head per 32-col PE group).
  - out-proj with head-strip-spread Wout + residual + LayerNorm.
Numerics: PSUM accumulation is fp32 everywhere; large streaming matmuls use
split-bf16 (hi/lo Dekker split, dropping only the lo*lo term -> ~2^-17
relative error, far below fp32 reference noise after softmax). The PV matmul
streams exp() scores in full fp32. No max-subtraction in softmax (scores are
O(1) so exp is well-conditioned); measured end-to-end rel err ~2e-6 vs the
fp32 jax reference.
"""

import sys

sys.path.insert(0, "/opt/trn_rl_repo")

import numpy as np

import concourse.bass as bass
import concourse.bacc as bacc
import concourse.tile as tile
from concourse import mybir
from concourse.bass_utils import run_bass_kernel_spmd

F32 = mybir.dt.float32
BF16 = mybir.dt.bfloat16
AF = mybir.ActivationFunctionType
ALU = mybir.AluOpType

B, T, N = 8, 16, 2048
HID, H, BOT, MAX_LAG = 32, 4, 8, 7
HD = HID // H  # 8
L = MAX_LAG + 1  # 8
EPS_LN = 1e-5
NCORES = 8

NJ = 4          # n-chunks of 512
NJW = 512
MI = 16         # m-chunks of 128
MIW = 128
NK = 16         # n-chunks of 128 for out-proj/LN
KC = 33         # contraction rows (32 features + ones row)
CPCOLS = 557    # packed-constants panel width

# dtype for the PV operands. float32r would stream at 1 cycle/row but is
# tf32-precision and cannot col-pack (dst partition must be 0) - keep fp32.
PV_DT = F32

_CACHE = {}


def _mm_cast(ap, dt):
    return ap if dt is None else ap.bitcast(dt)


def _build_program():
    if "nc" in _CACHE:
        return _CACHE["nc"]

    nc = bacc.Bacc("TRN2", target_bir_lowering=False, debug=False)

    # --- DRAM I/O ---
    xlhi_d = nc.dram_tensor("xlhi", [L, N], BF16, kind="ExternalInput")
    xllo_d = nc.dram_tensor("xllo", [L, N], BF16, kind="ExternalInput")
    cpk2_d = nc.dram_tensor("cpack2", [KC, 466], BF16, kind="ExternalInput")
    uthi_d = nc.dram_tensor("uthi", [H * BOT, N], BF16, kind="ExternalInput")
    utlo_d = nc.dram_tensor("utlo", [H * BOT, N], BF16, kind="ExternalInput")
    vnhi_d = nc.dram_tensor("vnhi", [H * BOT, N], BF16, kind="ExternalInput")
    vnlo_d = nc.dram_tensor("vnlo", [H * BOT, N], BF16, kind="ExternalInput")
    feat_d = nc.dram_tensor("feat", [N, HID], F32, kind="ExternalInput")
    featb_d = nc.dram_tensor("featb", [N, HID], F32, kind="ExternalInput")
    cpack_d = nc.dram_tensor("cpack", [128, CPCOLS], F32, kind="ExternalInput")
    out_d = nc.dram_tensor("out", [N, HID], F32, kind="ExternalOutput")

    with tile.TileContext(nc) as tc:
        with (
            tc.tile_pool(name="const", bufs=1) as cpool,
            tc.tile_pool(name="big", bufs=1) as bpool,
        ):
            # ---- all small constants arrive in ONE DMA ----
            cpk = cpool.tile([128, CPCOLS], F32)
            nc.sync.dma_start(cpk[:], cpack_d[:, :])
            wouts = cpk[:, 168:200]
            s2b = cpk[:, 200:328]
            ident = cpk[:, 330:458]
            gdb = cpk[0:HID, 458:459]
            dw8 = cpk[0:L, 459:460]
            gdw = cpk[0:1, 460:492]
            gbrow = cpk[0:1, 492:556]
            eps_t = cpk[:, 556:557]

            onescol = cpool.tile([1, 128], F32)
            nc.gpsimd.memset(onescol[:], 1.0)

            xlhi = cpool.tile([L, N], BF16)
            nc.sync.dma_start(xlhi[:], xlhi_d[:, :])
            xllo = cpool.tile([L, N], BF16)
            nc.sync.dma_start(xllo[:], xllo_d[:, :])
            cpk2 = cpool.tile([KC, 466], BF16)
            nc.sync.dma_start(cpk2[:], cpk2_d[:, :])
            wqh = cpk2[0:KC, 0:64]
            wql = cpk2[0:KC, 64:128]
            wkh = cpk2[0:KC, 128:192]
            wkl = cpk2[0:KC, 192:256]
            wvvh = cpk2[0:KC, 256:296]
            wvvl = cpk2[0:KC, 296:336]
            sel = cpk2[0:32, 336:400]
            dw8hi = cpk2[0:L, 400:401]
            dw8lo = cpk2[0:L, 401:402]
            gdwhi = cpk2[0:1, 402:434]
            gdwlo = cpk2[0:1, 434:466]

            uthi = cpool.tile([H * BOT, N], BF16)
            nc.gpsimd.dma_start(uthi[:], uthi_d[:, :])
            utlo = cpool.tile([H * BOT, N], BF16)
            nc.gpsimd.dma_start(utlo[:], utlo_d[:, :])
            vnhi = cpool.tile([H * BOT, N], BF16)
            nc.sync.dma_start(vnhi[:], vnhi_d[:, :])
            vnlo = cpool.tile([H * BOT, N], BF16)
            nc.sync.dma_start(vnlo[:], vnlo_d[:, :])

            # features (attention path) and features+out_bias (residual path)
            feat_sb = bpool.tile([128, NK * HID], F32)
            featv = feat_sb[:].rearrange("p (k c) -> p k c", k=NK)
            nc.sync.dma_start(featv, feat_d[:, :].rearrange("(k p) c -> p k c", p=128))
            featb_sb = bpool.tile([128, NK * HID], F32)
            featbv = featb_sb[:].rearrange("p (k c) -> p k c", k=NK)
            nc.gpsimd.dma_start(
                featbv, featb_d[:, :].rearrange("(k p) c -> p k c", p=128)
            )

            # ---- persistent big SBUF tensors ----
            qu = bpool.tile([128, N], F32)   # rows 32h+0:8 = q_h/sqrt(hd), +8:16 = u_h^T
            kv = bpool.tile([128, N], F32)   # rows 32h+0:8 = k_h,          +8:16 = v_h
            # bf16 hi/lo splits of qu/kv for single-pass split-bf16 QK matmuls
            quhi = bpool.tile([128, N], BF16)
            qulo = bpool.tile([128, N], BF16)
            kvhi = bpool.tile([128, N], BF16)
            kvlo = bpool.tile([128, N], BF16)
            combT = bpool.tile([KC, N], F32)  # combined^T with ones row 32
            combThi = bpool.tile([KC, N], BF16)
            combTlo = bpool.tile([KC, N], BF16)
            delayed = bpool.tile([1, N], F32)
            # v1 = [vv(8) | 1 | 0] per (h, mi); 4 separate tiles (4 m-chunks
            # each) so the main loop's first PV doesn't wait on all fills
            v1g = []
            for mg in range(MI // 4):
                t = bpool.tile([128, H * 4 * 10], PV_DT, tag=f"v1g{mg}")
                v1g.append(t[:].rearrange("p (h m n) -> p h m n", h=H, m=4))
            numer_sb = bpool.tile([128, N], F32)  # 4 head-strips of 10 rows
            attT2 = bpool.tile([128, N], F32)     # normalized numerators
            rd2 = bpool.tile([128, N], F32)       # recip denominators (bcast rows)
            cent_all = bpool.tile([128, NK * HID], F32)
            sq_all = bpool.tile([128, NK * HID], F32)
            mu16 = bpool.tile([128, NK], F32)
            var16 = bpool.tile([128, NK], F32)
            std16 = bpool.tile([128, NK], F32)
            rstd16 = bpool.tile([128, NK], F32)
            gbT = bpool.tile([128, 2 * HID], F32)   # gamma/beta broadcast rows
            out_all = bpool.tile([128, NK * HID], F32)

            nc.vector.memset(combT[KC - 1 : KC, :], 1.0)
            nc.vector.memset(combThi[KC - 1 : KC, :], 1.0)
            nc.vector.memset(combTlo[KC - 1 : KC, :], 0.0)


            # ---- phase A: delayed, combT, qu/kv assembly, v1 ----
            with tc.tile_pool(name="pa", bufs=8, space="PSUM") as pa:
                # PE keep-alive: consume each DMA'd tensor as it lands so the
                # HAM clock gate stays at full rate through the prologue
                def keepalive(src16):
                    wp = pa.tile([128, 128], F32, tag="pa")
                    nc.tensor.matmul(
                        wp[:], src16[:, 0:128], src16[:, 0:128],
                        start=True, stop=True,
                    )

                for _ in range(4):
                    keepalive(cpk[0:16, :])
                keepalive(feat_sb[0:16, :])
                keepalive(featb_sb[0:16, :])
                keepalive(uthi[0:16, :])
                keepalive(vnhi[0:16, :])

                # delayed[n] = sum_lag dw8[lag] * xl[lag, n], split-bf16
                dterms = ((dw8hi, xlhi), (dw8lo, xlhi), (dw8hi, xllo))
                for nj in range(NJ):
                    dp = pa.tile([1, NJW], F32, tag="pa")
                    for t, (dwt, xt) in enumerate(dterms):
                        nc.tensor.matmul(
                            dp[:],
                            dwt,
                            xt[:, bass.ts(nj, NJW)],
                            start=(t == 0),
                            stop=(t == 2),
                            skip_group_check=True,
                        )
                    nc.vector.tensor_copy(delayed[:, bass.ts(nj, NJW)], dp[:])
                # bf16 hi/lo of delayed for the outer-product accumulation
                dhi = cpool.tile([1, N], BF16)
                dlo = cpool.tile([1, N], BF16)
                nc.scalar.activation(dhi[:], delayed[:], AF.Copy)
                nc.vector.tensor_tensor(dlo[:], delayed[:], dhi[:], op=ALU.subtract)

                # gamma/beta broadcast to 128 partitions
                gbp = pa.tile([128, 2 * HID], F32, tag="pa")
                nc.tensor.matmul(gbp[:], onescol[:], gbrow[:], start=True, stop=True)
                nc.vector.tensor_copy(gbT[:], gbp[:])

                # combT = feat^T + gdw (outer) delayed  (+gdb via ACT bias)
                oterms = ((gdwhi, dhi), (gdwlo, dhi), (gdwhi, dlo))
                for nj in range(NJ):
                    ct = pa.tile([HID, NJW], F32, tag="pa")
                    for k in range(4):
                        nc.tensor.matmul(
                            ct[:, bass.ts(k, 128)],
                            featv[:, nj * 4 + k, :],
                            ident[:],
                            is_transpose=True,
                            start=(k == 0),
                            stop=False,
                            skip_group_check=True,
                        )
                    for t, (gw, dd) in enumerate(oterms):
                        nc.tensor.matmul(
                            ct[:],
                            gw,
                            dd[:, bass.ts(nj, NJW)],
                            start=False,
                            stop=(t == 2),
                            skip_group_check=True,
                        )
                    nc.scalar.activation(
                        combT[0:HID, bass.ts(nj, NJW)],
                        ct[:],
                        AF.Identity,
                        bias=gdb[:],
                    )
                    cs = bass.ts(nj, NJW)
                    nc.scalar.activation(
                        combThi[0:HID, cs], combT[0:HID, cs], AF.Copy
                    )
                    nc.vector.tensor_tensor(
                        combTlo[0:HID, cs], combT[0:HID, cs], combThi[0:HID, cs],
                        op=ALU.subtract,
                    )


                # assemble qu / kv: per head one matmul projects q (or k) AND
                # passes through u (or v) into a 16-row group at bp 32h;
                # bias-add + hi-copy on ACT, lo-sub on DVE (engine balance)
                for src, hi, lo, wh, wl, pu, pl in (
                    (qu, quhi, qulo, wqh, wql, uthi, utlo),
                    (kv, kvhi, kvlo, wkh, wkl, vnhi, vnlo),
                ):
                    for nj in range(NJ):
                        qp = pa.tile([128, NJW], F32, tag="pa")
                        nc.vector.memset(qp[:], 0.0)
                        s = bass.ts(nj, NJW)
                        terms = (
                            (wh, combThi[:, s]),
                            (wl, combThi[:, s]),
                            (wh, combTlo[:, s]),
                            (sel, pu[:, s]),
                            (sel, pl[:, s]),
                        )
                        for t, (wt, rt) in enumerate(terms):
                            for h in range(H):
                                nc.tensor.matmul(
                                    qp[32 * h : 32 * h + 16, :],
                                    wt[:, bass.ts(h, 16)],
                                    rt,
                                    start=(t == 0),
                                    stop=(t == len(terms) - 1),
                                    tile_position=(0, 32 * h),
                                    skip_group_check=True,
                                )
                        nc.scalar.activation(src[:, s], qp[:], AF.Copy)
                        nc.scalar.activation(hi[:, s], src[:, s], AF.Copy)
                        nc.vector.tensor_tensor(
                            lo[:, s], src[:, s], hi[:, s], op=ALU.subtract
                        )

                # vv natural [m, d] per m-chunk -> v1 strided; wvv's extra
                # columns synthesize the ones (and zero-pad) columns from the
                # combT ones row, so v1 = [vv(8) | 1 | 0] per (h, mi).
                # batched 4 m-chunks per psum tile to cut dependency hops
                for mg in range(MI // 4):
                    vp = pa.tile([128, 160], F32, tag="pa")
                    for k in range(4):
                        vterms = (
                            (combThi, wvvh),
                            (combThi, wvvl),
                            (combTlo, wvvh),
                        )
                        for t, (cb, wv) in enumerate(vterms):
                            nc.tensor.matmul(
                                vp[:, bass.ts(k, 40)],
                                cb[:, bass.ts(mg * 4 + k, MIW)],
                                wv,
                                start=(t == 0),
                                stop=(t == 2),
                                skip_group_check=True,
                            )
                    vpv = vp[:].rearrange("p (m c) -> p m c", m=4)
                    nc.vector.tensor_copy(
                        v1g[mg][:, :, :, 0:BOT],
                        vpv[:, :, 0:HID]
                        .rearrange("p m (h d) -> p h m d", h=H),
                    )
                    nc.vector.tensor_copy(
                        v1g[mg][:, :, :, BOT : BOT + 2],
                        vpv[:, :, HID:40]
                        .rearrange("p m (o h) -> p h m o", h=H),
                    )

            # ---- phase B: main attention loop (+ lazily interleaved epilogue) ----
            with (
                tc.tile_pool(name="qk", bufs=3, space="PSUM") as qkpool,
                tc.tile_pool(name="nm", bufs=1, space="PSUM") as nmpool,
                tc.tile_pool(name="dbop", bufs=1, space="PSUM") as dbpool,
                tc.tile_pool(name="es", bufs=6) as espool,
            ):
                def pv_quad(numer, pmi, pest):
                    for h in range(H):
                        nc.tensor.matmul(
                            numer[32 * h : 32 * h + 10, :],
                            v1g[pmi // 4][:, h, pmi % 4, :],
                            pest[h // 2][:, bass.ts(h % 2, NJW)],
                            start=(pmi == 0),
                            stop=(pmi == MI - 1),
                            tile_position=(0, 32 * h),
                            skip_group_check=True,
                        )

                # per-nj epilogue steps, emitted lazily between the next nj's
                # iterations so they fill PE/DVE slack instead of stalling ACT
                pending = []

                def flush(k=1):
                    for _ in range(min(k, len(pending))):
                        pending.pop(0)()

                def make_epilogue(nj, numer):
                    def s_evac():
                        nc.vector.tensor_copy(
                            numer_sb[:, bass.ts(nj, NJW)], numer[:]
                        )

                    def s_db():
                        db = dbpool.tile([128, NJW], F32, tag="dbop")
                        nc.tensor.matmul(
                            db[:],
                            s2b[:],
                            numer_sb[:, bass.ts(nj, NJW)],
                            start=True,
                            stop=True,
                        )
                        nc.vector.reciprocal(rd2[:, bass.ts(nj, NJW)], db[:])

                    def s_norm():
                        nc.vector.tensor_tensor(
                            attT2[:, bass.ts(nj, NJW)],
                            numer_sb[:, bass.ts(nj, NJW)],
                            rd2[:, bass.ts(nj, NJW)],
                            op=ALU.mult,
                        )

                    def s_oproj():
                        opj = dbpool.tile([128, 128], F32, tag="dbop")
                        for k in range(4):
                            nc.tensor.matmul(
                                opj[:, bass.ts(k, HID)],
                                attT2[:, bass.ts(nj * 4 + k, 128)],
                                wouts[:],
                                start=True,
                                stop=True,
                            )
                        nc.vector.tensor_tensor(
                            cent_all[:, bass.ts(nj, 4 * HID)].rearrange(
                                "p (m c) -> p m c", m=4
                            ),
                            opj[:].rearrange("p (m c) -> p m c", m=4),
                            featbv[:, nj * 4 : nj * 4 + 4, :],
                            op=ALU.add,
                        )

                    def s_ln1():
                        c3 = cent_all[:, bass.ts(nj, 4 * HID)].rearrange(
                            "p (m c) -> p m c", m=4
                        )
                        s3 = sq_all[:, bass.ts(nj, 4 * HID)].rearrange(
                            "p (m c) -> p m c", m=4
                        )
                        mu4 = mu16[:, nj * 4 : nj * 4 + 4]
                        nc.vector.tensor_reduce(
                            mu4, c3, axis=mybir.AxisListType.X, op=ALU.add
                        )
                        nc.vector.tensor_scalar_mul(mu4, mu4, 1.0 / HID)
                        mu3 = mu4.rearrange("p (m o) -> p m o", o=1).broadcast_to(
                            [128, 4, HID]
                        )
                        nc.vector.tensor_tensor(c3, c3, mu3, op=ALU.subtract)
                        nc.vector.tensor_tensor(s3, c3, c3, op=ALU.mult)
                        nc.vector.tensor_reduce(
                            var16[:, nj * 4 : nj * 4 + 4],
                            s3,
                            axis=mybir.AxisListType.X,
                            op=ALU.add,
                        )

                    return [s_evac, s_db, s_norm, s_oproj, s_ln1]

                for nj in range(NJ):
                    numer = nmpool.tile([128, NJW], F32, tag="nm")
                    nc.vector.memset(numer[:], 0.0)
                    prev = None
                    for mi in range(MI):
                        qka = qkpool.tile([128, 1024], F32, tag="qk")
                        qkb = qkpool.tile([128, 1024], F32, tag="qk")
                        qks = [qka, qkb]
                        for term, (lt, rt) in enumerate(
                            ((kvhi, quhi), (kvlo, quhi), (kvhi, qulo))
                        ):
                            for h in range(H):
                                nc.tensor.matmul(
                                    qks[h // 2][:, bass.ts(h % 2, NJW)],
                                    lt[32 * h : 32 * h + 16, bass.ts(mi, MIW)],
                                    rt[32 * h : 32 * h + 16, bass.ts(nj, NJW)],
                                    start=(term == 0),
                                    stop=(term == 2),
                                    tile_position=(32 * h, 0),
                                    skip_group_check=True,
                                )
                        est = []
                        for half in range(2):
                            es = espool.tile([128, 1024], PV_DT, tag="es")
                            nc.scalar.activation(es[:], qks[half][:], AF.Exp)
                            est.append(es)
                        # PV for the PREVIOUS iteration: keeps PV behind the
                        # next QK in PE program order so the exp->exp chain
                        # never transitively waits on a trailing PV quad.
                        if prev is not None:
                            pv_quad(numer, *prev)
                        prev = (mi, est)
                        flush(1)
                    pv_quad(numer, *prev)
                    pending.extend(make_epilogue(nj, numer))
                flush(len(pending))

                # ---- LN finish: one sqrt table switch, then scale + DMA ----
                nc.scalar.activation(
                    std16[:], var16[:], AF.Sqrt, bias=eps_t[:], scale=1.0 / HID
                )
                nc.vector.reciprocal(rstd16[:], std16[:])
                c3 = cent_all[:].rearrange("p (m c) -> p m c", m=NK)
                o3 = out_all[:].rearrange("p (m c) -> p m c", m=NK)
                r3 = rstd16[:].rearrange("p (m o) -> p m o", o=1).broadcast_to(
                    [128, NK, HID]
                )
                g3 = gbT[:, 0:HID].rearrange("p (o c) -> p o c", o=1).broadcast_to(
                    [128, NK, HID]
                )
                b3 = gbT[:, HID : 2 * HID].rearrange(
                    "p (o c) -> p o c", o=1
                ).broadcast_to([128, NK, HID])
                nc.vector.tensor_tensor(o3, c3, r3, op=ALU.mult)
                nc.vector.tensor_tensor(o3, o3, g3, op=ALU.mult)
                nc.vector.tensor_tensor(o3, o3, b3, op=ALU.add)
                nc.sync.dma_start(
                    out_d[:, :].rearrange("(k p) c -> p k c", p=128),
                    out_all[:].rearrange("p (k c) -> p k c", k=NK),
                )

    nc.finalize()
    _CACHE["nc"] = nc
    return nc


def _host_prep(inputs):
    """Host-side prep of small parameters + per-core sharding of big inputs."""
    f32 = np.float32
    x = np.asarray(inputs["x"], f32)
    features = np.asarray(inputs["features"], f32)
    u = np.asarray(inputs["u"], f32)
    v = np.asarray(inputs["v"], f32)

    dl = np.asarray(inputs["delay_logits"], f32)
    e = np.exp(dl - dl.max())
    dw = (e / e.sum()).astype(f32)[:L]
    gate = f32(1.0 / (1.0 + np.exp(-f32(inputs["delay_gate"]))))
    gdw = (gate * np.asarray(inputs["delay_w"], f32)[0]).astype(f32)
    gdb = (gate * np.asarray(inputs["delay_b"], f32)).astype(f32)

    w1 = (np.asarray(inputs["qkv_low_w"], f32) @ np.asarray(inputs["qkv_high_w"], f32)).astype(f32)
    b1 = (np.asarray(inputs["qkv_low_b"], f32) @ np.asarray(inputs["qkv_high_w"], f32)
          + np.asarray(inputs["qkv_high_b"], f32)).astype(f32)
    sc = f32(1.0 / np.sqrt(HD))
    w1 = w1.copy()
    b1 = b1.copy()
    w1[:, :HID] *= sc
    b1[:HID] *= sc

    # fused q/u (and k/v) assembly weights: [96, 16] per head, 4 heads packed
    wuq = np.zeros((96, 64), f32)
    wuk = np.zeros((96, 64), f32)
    for h in range(H):
        wuq[0:32, 16 * h : 16 * h + 8] = w1[:, HD * h : HD * h + HD]
        wuk[0:32, 16 * h : 16 * h + 8] = w1[:, HID + HD * h : HID + HD * h + HD]
        for r in range(BOT):
            wuq[32 + BOT * h + r, 16 * h + 8 + r] = 1.0
            wuk[64 + BOT * h + r, 16 * h + 8 + r] = 1.0

    qbias = np.zeros((128, 1), f32)
    kbias = np.zeros((128, 1), f32)
    for h in range(H):
        qbias[32 * h : 32 * h + 8, 0] = b1[HD * h : HD * h + HD]
        kbias[32 * h : 32 * h + 8, 0] = b1[HID + HD * h : HID + HD * h + HD]

    # [33, 40]: cols 0:32 vv projection (+bias row), cols 32+h = ones column
    # per head (coefficient 1 on the combT ones row), cols 36+h = zeros
    wvv = np.zeros((KC, 40), f32)
    wvv[0:32, 0:HID] = w1[:, 2 * HID :]
    wvv[32, 0:HID] = b1[2 * HID :]
    for h in range(H):
        wvv[32, HID + h] = 1.0

    wo = (np.asarray(inputs["out_low_w"], f32) @ np.asarray(inputs["out_high_w"], f32)).astype(f32)
    bo = (np.asarray(inputs["out_low_b"], f32) @ np.asarray(inputs["out_high_w"], f32)
          + np.asarray(inputs["out_high_b"], f32)).astype(f32)
    wouts = np.zeros((128, HID), f32)
    for h in range(H):
        wouts[32 * h : 32 * h + 8, :] = wo[HD * h : HD * h + HD, :]

    s2b = np.zeros((128, 128), f32)
    for h in range(H):
        s2b[32 * h + 8, 32 * h : 32 * h + 32] = 1.0

    gb = np.concatenate(
        [np.asarray(inputs["ln_gamma"], f32), np.asarray(inputs["ln_beta"], f32)]
    )[None, :].astype(f32)

    cpack = np.zeros((128, CPCOLS), f32)
    cpack[0:96, 0:64] = wuq
    cpack[0:96, 64:128] = wuk
    cpack[0:KC, 128:168] = wvv
    cpack[:, 168:200] = wouts
    cpack[:, 200:328] = s2b
    cpack[:, 328:329] = qbias
    cpack[:, 329:330] = kbias
    cpack[:, 330:458] = np.eye(128, dtype=f32)
    cpack[0:HID, 458] = gdb
    cpack[0:L, 459] = dw
    cpack[0, 460:492] = gdw
    cpack[0, 492:556] = gb[0]
    cpack[:, 556] = EPS_LN

    import ml_dtypes

    bf16 = ml_dtypes.bfloat16

    def split(a):
        hi = a.astype(bf16)
        return hi, (a - hi.astype(f32)).astype(bf16)

    # bf16 weight panel [33, 466]
    wq33 = np.zeros((KC, 64), f32)
    wk33 = np.zeros((KC, 64), f32)
    selm = np.zeros((32, 64), f32)
    for h in range(H):
        wq33[0:32, 16 * h : 16 * h + 8] = w1[:, HD * h : HD * h + HD]
        wq33[32, 16 * h : 16 * h + 8] = b1[HD * h : HD * h + HD]
        wk33[0:32, 16 * h : 16 * h + 8] = w1[:, HID + HD * h : HID + HD * h + HD]
        wk33[32, 16 * h : 16 * h + 8] = b1[HID + HD * h : HID + HD * h + HD]
        for r in range(BOT):
            selm[BOT * h + r, 16 * h + 8 + r] = 1.0
    cpack2 = np.zeros((KC, 466), bf16)
    cpack2[:, 0:64], cpack2[:, 64:128] = split(wq33)
    cpack2[:, 128:192], cpack2[:, 192:256] = split(wk33)
    cpack2[:, 256:296], cpack2[:, 296:336] = split(wvv)
    cpack2[0:32, 336:400] = selm.astype(bf16)
    dwhi, dwlo = split(dw)
    cpack2[0:L, 400] = dwhi
    cpack2[0:L, 401] = dwlo
    gdwhi, gdwlo = split(gdw)
    cpack2[0, 402:434] = gdwhi
    cpack2[0, 434:466] = gdwlo

    ut = np.ascontiguousarray(u.transpose(0, 2, 1).reshape(H * BOT, N))
    vnn = np.ascontiguousarray(v.reshape(H * BOT, N))
    uthi, utlo = split(ut)
    vnhi, vnlo = split(vnn)

    common = {
        "cpack": cpack,
        "cpack2": cpack2,
        "uthi": uthi, "utlo": utlo, "vnhi": vnhi, "vnlo": vnlo,
    }

    xl = np.ascontiguousarray(x[:, T - 1 : T - 1 - L : -1, :])  # [B, L, N]
    xlhi = xl.astype(bf16)
    xllo = (xl - xlhi.astype(f32)).astype(bf16)

    in_maps = []
    for b in range(B):
        m = dict(common)
        m["xlhi"] = np.ascontiguousarray(xlhi[b])
        m["xllo"] = np.ascontiguousarray(xllo[b])
        m["feat"] = np.ascontiguousarray(features[b])
        m["featb"] = np.ascontiguousarray(features[b] + bo[None, :]).astype(f32)
        in_maps.append(m)

    loss = f32(np.exp(np.asarray(inputs["log_attn_reg"], f32)) / f32(N))
    return in_maps, loss


def kernel(**inputs):
    nc = _build_program()
    in_maps, loss = _host_prep(inputs)
    res = run_bass_kernel_spmd(nc, in_maps, core_ids=list(range(NCORES)))
    out = np.stack([res.results[b]["out"] for b in range(B)], axis=0)
    return out.astype(np.float32), loss


# revision 66
# speedup vs baseline: 1.2385x; 1.2385x over previous
"""Trainium2 Bass kernel for nn_EpiSIGNetV5 (sparse_attention).

Strategy: data-parallel over batch B=8 across the 8 NeuronCores (1 batch each).
Per core, the whole network is fused on-chip:
  - delayed signal + combined features built transposed ([33, N] incl. a ones
    row so matmul biases ride along in the contraction)
  - QKV via a fused projection (low@high collapsed on host); q&u / k&v are
    assembled into 16-row head groups by a single matmul per head that both
    projects (W block) and passes through u/v (identity block)
  - attention scores computed TRANSPOSED ([m, n] tiles) with the graph bias
    u@v folded into the same K=16 contraction; exp() runs straight out of
    PSUM on the scalar engine; the softmax denominator comes from a ones
    column appended to V in the PV matmul, so no transposes and no
    partition-dim reductions are ever needed.
  - QK matmuls are 4-way row-group packed (one head per 32-row PE group);
    PV matmuls are 4-way col-group packed (one head per 32-col PE group).
  - out-proj with head-strip-spread Wout + residual + LayerNorm.
Numerics: PSUM accumulation is fp32 everywhere; large streaming matmuls use
split-bf16 (hi/lo Dekker split, dropping only the lo*lo term -> ~2^-17
relative error, far below fp32 reference noise after softmax). The PV matmul
streams exp() scores in full fp32. No max-subtraction in softmax (scores are
O(1) so exp is well-conditioned); measured end-to-end rel err ~2e-6 vs the
fp32 jax reference.
"""

import sys

sys.path.insert(0, "/opt/trn_rl_repo")

import numpy as np

import concourse.bass as bass
import concourse.bacc as bacc
import concourse.tile as tile
from concourse import mybir
from concourse.bass_utils import run_bass_kernel_spmd

F32 = mybir.dt.float32
BF16 = mybir.dt.bfloat16
AF = mybir.ActivationFunctionType
ALU = mybir.AluOpType

B, T, N = 8, 16, 2048
HID, H, BOT, MAX_LAG = 32, 4, 8, 7
HD = HID // H  # 8
L = MAX_LAG + 1  # 8
EPS_LN = 1e-5
NCORES = 8

NJ = 4          # n-chunks of 512
NJW = 512
MI = 16         # m-chunks of 128
MIW = 128
NK = 16         # n-chunks of 128 for out-proj/LN
KC = 33         # contraction rows (32 features + ones row)
CPCOLS = 557    # packed-constants panel width

# dtype for the PV operands. float32r would stream at 1 cycle/row but is
# tf32-precision and cannot col-pack (dst partition must be 0) - keep fp32.
PV_DT = F32

_CACHE = {}


def _mm_cast(ap, dt):
    return ap if dt is None else ap.bitcast(dt)


def _build_program():
    if "nc" in _CACHE:
        return _CACHE["nc"]

    nc = bacc.Bacc("TRN2", target_bir_lowering=False, debug=False)

    # --- DRAM I/O ---
    xlhi_d = nc.dram_tensor("xlhi", [L, N], BF16, kind="ExternalInput")
    xllo_d = nc.dram_tensor("xllo", [L, N], BF16, kind="ExternalInput")
    cpk2_d = nc.dram_tensor("cpack2", [KC, 466], BF16, kind="ExternalInput")
    uthi_d = nc.dram_tensor("uthi", [H * BOT, N], BF16, kind="ExternalInput")
    utlo_d = nc.dram_tensor("utlo", [H * BOT, N], BF16, kind="ExternalInput")
    vnhi_d = nc.dram_tensor("vnhi", [H * BOT, N], BF16, kind="ExternalInput")
    vnlo_d = nc.dram_tensor("vnlo", [H * BOT, N], BF16, kind="ExternalInput")
    feat_d = nc.dram_tensor("feat", [N, HID], F32, kind="ExternalInput")
    featb_d = nc.dram_tensor("featb", [N, HID], F32, kind="ExternalInput")
    cpack_d = nc.dram_tensor("cpack", [128, CPCOLS], F32, kind="ExternalInput")
    out_d = nc.dram_tensor("out", [N, HID], F32, kind="ExternalOutput")

    with tile.TileContext(nc) as tc:
        with (
            tc.tile_pool(name="const", bufs=1) as cpool,
            tc.tile_pool(name="es", bufs=6) as espool,
            tc.tile_pool(name="big", bufs=1) as bpool,
        ):
            # ---- all small constants arrive in ONE DMA ----
            cpk = cpool.tile([128, CPCOLS], F32)
            nc.sync.dma_start(cpk[:], cpack_d[:, :])
            wouts = cpk[:, 168:200]
            s2b = cpk[:, 200:328]
            ident = cpk[:, 330:458]
            gdb = cpk[0:HID, 458:459]
            dw8 = cpk[0:L, 459:460]
            gdw = cpk[0:1, 460:492]
            gbrow = cpk[0:1, 492:556]
            eps_t = cpk[:, 556:557]

            onescol = cpool.tile([1, 128], F32)
            nc.gpsimd.memset(onescol[:], 1.0)

            xlhi = cpool.tile([L, N], BF16)
            nc.sync.dma_start(xlhi[:], xlhi_d[:, :])
            xllo = cpool.tile([L, N], BF16)
            nc.sync.dma_start(xllo[:], xllo_d[:, :])
            cpk2 = cpool.tile([KC, 466], BF16)
            nc.sync.dma_start(cpk2[:], cpk2_d[:, :])
            wqh = cpk2[0:KC, 0:64]
            wql = cpk2[0:KC, 64:128]
            wkh = cpk2[0:KC, 128:192]
            wkl = cpk2[0:KC, 192:256]
            wvvh = cpk2[0:KC, 256:296]
            wvvl = cpk2[0:KC, 296:336]
            sel = cpk2[0:32, 336:400]
            dw8hi = cpk2[0:L, 400:401]
            dw8lo = cpk2[0:L, 401:402]
            gdwhi = cpk2[0:1, 402:434]
            gdwlo = cpk2[0:1, 434:466]

            uthi = cpool.tile([H * BOT, N], BF16)
            nc.gpsimd.dma_start(uthi[:], uthi_d[:, :])
            utlo = cpool.tile([H * BOT, N], BF16)
            nc.gpsimd.dma_start(utlo[:], utlo_d[:, :])
            vnhi = cpool.tile([H * BOT, N], BF16)
            nc.sync.dma_start(vnhi[:], vnhi_d[:, :])
            vnlo = cpool.tile([H * BOT, N], BF16)
            nc.sync.dma_start(vnlo[:], vnlo_d[:, :])

            # features (attention path) and features+out_bias (residual path)
            feat_sb = bpool.tile([128, NK * HID], F32)
            featv = feat_sb[:].rearrange("p (k c) -> p k c", k=NK)
            nc.sync.dma_start(featv, feat_d[:, :].rearrange("(k p) c -> p k c", p=128))
            featb_sb = bpool.tile([128, NK * HID], F32)
            featbv = featb_sb[:].rearrange("p (k c) -> p k c", k=NK)
            nc.gpsimd.dma_start(
                featbv, featb_d[:, :].rearrange("(k p) c -> p k c", p=128)
            )

            # ---- persistent big SBUF tensors ----
            qu = bpool.tile([128, N], F32)   # rows 32h+0:8 = q_h/sqrt(hd), +8:16 = u_h^T
            kv = bpool.tile([128, N], F32)   # rows 32h+0:8 = k_h,          +8:16 = v_h
            # bf16 hi/lo splits of qu/kv for single-pass split-bf16 QK matmuls
            quhi = bpool.tile([128, N], BF16)
            qulo = bpool.tile([128, N], BF16)
            kvhi = bpool.tile([128, N], BF16)
            kvlo = bpool.tile([128, N], BF16)
            combT = bpool.tile([KC, N], F32)  # combined^T with ones row 32
            combThi = bpool.tile([KC, N], BF16)
            combTlo = bpool.tile([KC, N], BF16)
            delayed = bpool.tile([1, N], F32)
            # v1 = [vv(8) | 1 | 0] per (h, mi); 4 separate tiles (4 m-chunks
            # each) so the main loop's first PV doesn't wait on all fills
            v1g = []
            for mg in range(MI // 4):
                t = bpool.tile([128, H * 4 * 10], PV_DT, tag=f"v1g{mg}")
                v1g.append(t[:].rearrange("p (h m n) -> p h m n", h=H, m=4))
            numer_sb = bpool.tile([128, N], F32)  # 4 head-strips of 10 rows
            attT2 = bpool.tile([128, N], F32)     # normalized numerators
            rd2 = bpool.tile([128, N], F32)       # recip denominators (bcast rows)
            cent_all = bpool.tile([128, NK * HID], F32)
            sq_all = bpool.tile([128, NK * HID], F32)
            mu16 = bpool.tile([128, NK], F32)
            var16 = bpool.tile([128, NK], F32)
            std16 = bpool.tile([128, NK], F32)
            rstd16 = bpool.tile([128, NK], F32)
            gbT = bpool.tile([128, 2 * HID], F32)   # gamma/beta broadcast rows
            out_all = bpool.tile([128, NK * HID], F32)

            nc.vector.memset(combT[KC - 1 : KC, :], 1.0)
            nc.vector.memset(combThi[KC - 1 : KC, :], 1.0)
            nc.vector.memset(combTlo[KC - 1 : KC, :], 0.0)


            # ---- phase A: delayed, combT, qu/kv assembly, v1 ----
            with tc.tile_pool(name="pa", bufs=8, space="PSUM") as pa:
                # PE keep-alive: consume each DMA'd tensor as it lands so the
                # HAM clock gate stays at full rate through the prologue
                def keepalive(src16):
                    wp = pa.tile([128, 128], F32, tag="pa")
                    nc.tensor.matmul(
                        wp[:], src16[:, 0:128], src16[:, 0:128],
                        start=True, stop=True,
                    )

                for _ in range(4):
                    keepalive(cpk[0:16, :])
                keepalive(feat_sb[0:16, :])
                keepalive(featb_sb[0:16, :])
                keepalive(uthi[0:16, :])
                keepalive(vnhi[0:16, :])

                # delayed[n] = sum_lag dw8[lag] * xl[lag, n], split-bf16
                dterms = ((dw8hi, xlhi), (dw8lo, xlhi), (dw8hi, xllo))
                for nj in range(NJ):
                    dp = pa.tile([1, NJW], F32, tag="pa")
                    for t, (dwt, xt) in enumerate(dterms):
                        nc.tensor.matmul(
                            dp[:],
                            dwt,
                            xt[:, bass.ts(nj, NJW)],
                            start=(t == 0),
                            stop=(t == 2),
                            skip_group_check=True,
                        )
                    nc.vector.tensor_copy(delayed[:, bass.ts(nj, NJW)], dp[:])
                # bf16 hi/lo of delayed for the outer-product accumulation
                dhi = cpool.tile([1, N], BF16)
                dlo = cpool.tile([1, N], BF16)
                nc.scalar.activation(dhi[:], delayed[:], AF.Copy)
                nc.vector.tensor_tensor(dlo[:], delayed[:], dhi[:], op=ALU.subtract)

                # gamma/beta broadcast to 128 partitions
                gbp = pa.tile([128, 2 * HID], F32, tag="pa")
                nc.tensor.matmul(gbp[:], onescol[:], gbrow[:], start=True, stop=True)
                nc.vector.tensor_copy(gbT[:], gbp[:])

                # combT = feat^T + gdw (outer) delayed  (+gdb via ACT bias)
                oterms = ((gdwhi, dhi), (gdwlo, dhi), (gdwhi, dlo))
                for nj in range(NJ):
                    ct = pa.tile([HID, NJW], F32, tag="pa")
                    for k in range(4):
                        nc.tensor.matmul(
                            ct[:, bass.ts(k, 128)],
                            featv[:, nj * 4 + k, :],
                            ident[:],
                            is_transpose=True,
                            start=(k == 0),
                            stop=False,
                            skip_group_check=True,
                        )
                    for t, (gw, dd) in enumerate(oterms):
                        nc.tensor.matmul(
                            ct[:],
                            gw,
                            dd[:, bass.ts(nj, NJW)],
                            start=False,
                            stop=(t == 2),
                            skip_group_check=True,
                        )
                    nc.scalar.activation(
                        combT[0:HID, bass.ts(nj, NJW)],
                        ct[:],
                        AF.Identity,
                        bias=gdb[:],
                    )
                    cs = bass.ts(nj, NJW)
                    nc.scalar.activation(
                        combThi[0:HID, cs], combT[0:HID, cs], AF.Copy
                    )
                    nc.vector.tensor_tensor(
                        combTlo[0:HID, cs], combT[0:HID, cs], combThi[0:HID, cs],
                        op=ALU.subtract,
                    )


                # assemble qu / kv: per head one matmul projects q (or k) AND
                # passes through u (or v) into a 16-row group at bp 32h;
                # bias-add + hi-copy on ACT, lo-sub on DVE (engine balance)
                for src, hi, lo, wh, wl, pu, pl in (
                    (qu, quhi, qulo, wqh, wql, uthi, utlo),
                    (kv, kvhi, kvlo, wkh, wkl, vnhi, vnlo),
                ):
                    for nj in range(NJ):
                        qp = pa.tile([128, NJW], F32, tag="pa")
                        nc.vector.memset(qp[:], 0.0)
                        s = bass.ts(nj, NJW)
                        terms = (
                            (wh, combThi[:, s]),
                            (wl, combThi[:, s]),
                            (wh, combTlo[:, s]),
                            (sel, pu[:, s]),
                            (sel, pl[:, s]),
                        )
                        for t, (wt, rt) in enumerate(terms):
                            for h in range(H):
                                nc.tensor.matmul(
                                    qp[32 * h : 32 * h + 16, :],
                                    wt[:, bass.ts(h, 16)],
                                    rt,
                                    start=(t == 0),
                                    stop=(t == len(terms) - 1),
                                    tile_position=(0, 32 * h),
                                    skip_group_check=True,
                                )
                        nc.scalar.activation(src[:, s], qp[:], AF.Copy)
                        nc.scalar.activation(hi[:, s], src[:, s], AF.Copy)
                        nc.vector.tensor_tensor(
                            lo[:, s], src[:, s], hi[:, s], op=ALU.subtract
                        )

                # vv natural [m, d] per m-chunk -> v1 strided; wvv's extra
                # columns synthesize the ones (and zero-pad) columns from the
                # combT ones row, so v1 = [vv(8) | 1 | 0] per (h, mi).
                # batched 4 m-chunks per psum tile to cut dependency hops
                for mg in range(MI // 4):
                    vp = pa.tile([128, 160], F32, tag="pa")
                    for k in range(4):
                        vterms = (
                            (combThi, wvvh),
                            (combThi, wvvl),
                            (combTlo, wvvh),
                        )
                        for t, (cb, wv) in enumerate(vterms):
                            nc.tensor.matmul(
                                vp[:, bass.ts(k, 40)],
                                cb[:, bass.ts(mg * 4 + k, MIW)],
                                wv,
                                start=(t == 0),
                                stop=(t == 2),
                                skip_group_check=True,
                            )
                    vpv = vp[:].rearrange("p (m c) -> p m c", m=4)
                    nc.vector.tensor_copy(
                        v1g[mg][:, :, :, 0:BOT],
                        vpv[:, :, 0:HID]
                        .rearrange("p m (h d) -> p h m d", h=H),
                    )
                    nc.vector.tensor_copy(
                        v1g[mg][:, :, :, BOT : BOT + 2],
                        vpv[:, :, HID:40]
                        .rearrange("p m (o h) -> p h m o", h=H),
                    )

            # ---- phase B: main attention loop (+ lazily interleaved epilogue) ----
            with (
                tc.tile_pool(name="qk", bufs=3, space="PSUM") as qkpool,
                tc.tile_pool(name="nm", bufs=1, space="PSUM") as nmpool,
                tc.tile_pool(name="dbop", bufs=1, space="PSUM") as dbpool,
            ):
                def pv_quad(numer, pmi, pest):
                    for h in range(H):
                        nc.tensor.matmul(
                            numer[32 * h : 32 * h + 10, :],
                            v1g[pmi // 4][:, h, pmi % 4, :],
                            pest[h // 2][:, bass.ts(h % 2, NJW)],
                            start=(pmi == 0),
                            stop=(pmi == MI - 1),
                            tile_position=(0, 32 * h),
                            skip_group_check=True,
                        )

                # per-nj epilogue steps, emitted lazily between the next nj's
                # iterations so they fill PE/DVE slack instead of stalling ACT
                pending = []

                def flush(k=1):
                    for _ in range(min(k, len(pending))):
                        pending.pop(0)()

                def make_epilogue(nj, numer):
                    def s_evac():
                        nc.vector.tensor_copy(
                            numer_sb[:, bass.ts(nj, NJW)], numer[:]
                        )

                    def s_db():
                        db = dbpool.tile([128, NJW], F32, tag="dbop")
                        nc.tensor.matmul(
                            db[:],
                            s2b[:],
                            numer_sb[:, bass.ts(nj, NJW)],
                            start=True,
                            stop=True,
                        )
                        nc.vector.reciprocal(rd2[:, bass.ts(nj, NJW)], db[:])

                    def s_norm():
                        nc.vector.tensor_tensor(
                            attT2[:, bass.ts(nj, NJW)],
                            numer_sb[:, bass.ts(nj, NJW)],
                            rd2[:, bass.ts(nj, NJW)],
                            op=ALU.mult,
                        )

                    def s_oproj():
                        opj = dbpool.tile([128, 128], F32, tag="dbop")
                        for k in range(4):
                            nc.tensor.matmul(
                                opj[:, bass.ts(k, HID)],
                                attT2[:, bass.ts(nj * 4 + k, 128)],
                                wouts[:],
                                start=True,
                                stop=True,
                            )
                        nc.vector.tensor_tensor(
                            cent_all[:, bass.ts(nj, 4 * HID)].rearrange(
                                "p (m c) -> p m c", m=4
                            ),
                            opj[:].rearrange("p (m c) -> p m c", m=4),
                            featbv[:, nj * 4 : nj * 4 + 4, :],
                            op=ALU.add,
                        )

                    def s_ln1():
                        c3 = cent_all[:, bass.ts(nj, 4 * HID)].rearrange(
                            "p (m c) -> p m c", m=4
                        )
                        s3 = sq_all[:, bass.ts(nj, 4 * HID)].rearrange(
                            "p (m c) -> p m c", m=4
                        )
                        mu4 = mu16[:, nj * 4 : nj * 4 + 4]
                        nc.vector.tensor_reduce(
                            mu4, c3, axis=mybir.AxisListType.X, op=ALU.add
                        )
                        nc.vector.tensor_scalar_mul(mu4, mu4, 1.0 / HID)
                        mu3 = mu4.rearrange("p (m o) -> p m o", o=1).broadcast_to(
                            [128, 4, HID]
                        )
                        nc.vector.tensor_tensor(c3, c3, mu3, op=ALU.subtract)
                        nc.vector.tensor_tensor(s3, c3, c3, op=ALU.mult)
                        nc.vector.tensor_reduce(
                            var16[:, nj * 4 : nj * 4 + 4],
                            s3,
                            axis=mybir.AxisListType.X,
                            op=ALU.add,
                        )

                    return [s_evac, s_db, s_norm, s_oproj, s_ln1]

                for nj in range(NJ):
                    numer = nmpool.tile([128, NJW], F32, tag="nm")
                    nc.vector.memset(numer[:], 0.0)
                    prev = None
                    for mi in range(MI):
                        qka = qkpool.tile([128, 1024], F32, tag="qk")
                        qkb = qkpool.tile([128, 1024], F32, tag="qk")
                        qks = [qka, qkb]
                        for term, (lt, rt) in enumerate(
                            ((kvhi, quhi), (kvlo, quhi), (kvhi, qulo))
                        ):
                            for h in range(H):
                                nc.tensor.matmul(
                                    qks[h // 2][:, bass.ts(h % 2, NJW)],
                                    lt[32 * h : 32 * h + 16, bass.ts(mi, MIW)],
                                    rt[32 * h : 32 * h + 16, bass.ts(nj, NJW)],
                                    start=(term == 0),
                                    stop=(term == 2),
                                    tile_position=(32 * h, 0),
                                    skip_group_check=True,
                                )
                        est = []
                        for half in range(2):
                            es = espool.tile([128, 1024], PV_DT, tag="es")
                            nc.scalar.activation(es[:], qks[half][:], AF.Exp)
                            est.append(es)
                        # PV for the PREVIOUS iteration: keeps PV behind the
                        # next QK in PE program order so the exp->exp chain
                        # never transitively waits on a trailing PV quad.
                        if prev is not None:
                            pv_quad(numer, *prev)
                        prev = (mi, est)
                        flush(1)
                    lastprev = prev
                    pending.append(lambda n=numer, p=lastprev: pv_quad(n, *p))
                    pending.extend(make_epilogue(nj, numer))
                flush(len(pending))

                # ---- LN finish: one sqrt table switch, then scale + DMA ----
                nc.scalar.activation(
                    std16[:], var16[:], AF.Sqrt, bias=eps_t[:], scale=1.0 / HID
                )
                nc.vector.reciprocal(rstd16[:], std16[:])
                c3 = cent_all[:].rearrange("p (m c) -> p m c", m=NK)
                o3 = out_all[:].rearrange("p (m c) -> p m c", m=NK)
                r3 = rstd16[:].rearrange("p (m o) -> p m o", o=1).broadcast_to(
                    [128, NK, HID]
                )
                g3 = gbT[:, 0:HID].rearrange("p (o c) -> p o c", o=1).broadcast_to(
                    [128, NK, HID]
                )
                b3 = gbT[:, HID : 2 * HID].rearrange(
                    "p (o c) -> p o c", o=1
                ).broadcast_to([128, NK, HID])
                nc.vector.tensor_tensor(o3, c3, r3, op=ALU.mult)
                nc.vector.tensor_tensor(o3, o3, g3, op=ALU.mult)
                nc.vector.tensor_tensor(o3, o3, b3, op=ALU.add)
                nc.sync.dma_start(
                    out_d[:, :].rearrange("(k p) c -> p k c", p=128),
                    out_all[:].rearrange("p (k c) -> p k c", k=NK),
                )

    nc.finalize()
    _CACHE["nc"] = nc
    return nc


def _host_prep(inputs):
    """Host-side prep of small parameters + per-core sharding of big inputs."""
    f32 = np.float32
    x = np.asarray(inputs["x"], f32)
    features = np.asarray(inputs["features"], f32)
    u = np.asarray(inputs["u"], f32)
    v = np.asarray(inputs["v"], f32)

    dl = np.asarray(inputs["delay_logits"], f32)
    e = np.exp(dl - dl.max())
    dw = (e / e.sum()).astype(f32)[:L]
    gate = f32(1.0 / (1.0 + np.exp(-f32(inputs["delay_gate"]))))
    gdw = (gate * np.asarray(inputs["delay_w"], f32)[0]).astype(f32)
    gdb = (gate * np.asarray(inputs["delay_b"], f32)).astype(f32)

    w1 = (np.asarray(inputs["qkv_low_w"], f32) @ np.asarray(inputs["qkv_high_w"], f32)).astype(f32)
    b1 = (np.asarray(inputs["qkv_low_b"], f32) @ np.asarray(inputs["qkv_high_w"], f32)
          + np.asarray(inputs["qkv_high_b"], f32)).astype(f32)
    sc = f32(1.0 / np.sqrt(HD))
    w1 = w1.copy()
    b1 = b1.copy()
    w1[:, :HID] *= sc
    b1[:HID] *= sc

    # fused q/u (and k/v) assembly weights: [96, 16] per head, 4 heads packed
    wuq = np.zeros((96, 64), f32)
    wuk = np.zeros((96, 64), f32)
    for h in range(H):
        wuq[0:32, 16 * h : 16 * h + 8] = w1[:, HD * h : HD * h + HD]
        wuk[0:32, 16 * h : 16 * h + 8] = w1[:, HID + HD * h : HID + HD * h + HD]
        for r in range(BOT):
            wuq[32 + BOT * h + r, 16 * h + 8 + r] = 1.0
            wuk[64 + BOT * h + r, 16 * h + 8 + r] = 1.0

    qbias = np.zeros((128, 1), f32)
    kbias = np.zeros((128, 1), f32)
    for h in range(H):
        qbias[32 * h : 32 * h + 8, 0] = b1[HD * h : HD * h + HD]
        kbias[32 * h : 32 * h + 8, 0] = b1[HID + HD * h : HID + HD * h + HD]

    # [33, 40]: cols 0:32 vv projection (+bias row), cols 32+h = ones column
    # per head (coefficient 1 on the combT ones row), cols 36+h = zeros
    wvv = np.zeros((KC, 40), f32)
    wvv[0:32, 0:HID] = w1[:, 2 * HID :]
    wvv[32, 0:HID] = b1[2 * HID :]
    for h in range(H):
        wvv[32, HID + h] = 1.0

    wo = (np.asarray(inputs["out_low_w"], f32) @ np.asarray(inputs["out_high_w"], f32)).astype(f32)
    bo = (np.asarray(inputs["out_low_b"], f32) @ np.asarray(inputs["out_high_w"], f32)
          + np.asarray(inputs["out_high_b"], f32)).astype(f32)
    wouts = np.zeros((128, HID), f32)
    for h in range(H):
        wouts[32 * h : 32 * h + 8, :] = wo[HD * h : HD * h + HD, :]

    s2b = np.zeros((128, 128), f32)
    for h in range(H):
        s2b[32 * h + 8, 32 * h : 32 * h + 32] = 1.0

    gb = np.concatenate(
        [np.asarray(inputs["ln_gamma"], f32), np.asarray(inputs["ln_beta"], f32)]
    )[None, :].astype(f32)

    cpack = np.zeros((128, CPCOLS), f32)
    cpack[0:96, 0:64] = wuq
    cpack[0:96, 64:128] = wuk
    cpack[0:KC, 128:168] = wvv
    cpack[:, 168:200] = wouts
    cpack[:, 200:328] = s2b
    cpack[:, 328:329] = qbias
    cpack[:, 329:330] = kbias
    cpack[:, 330:458] = np.eye(128, dtype=f32)
    cpack[0:HID, 458] = gdb
    cpack[0:L, 459] = dw
    cpack[0, 460:492] = gdw
    cpack[0, 492:556] = gb[0]
    cpack[:, 556] = EPS_LN

    import ml_dtypes

    bf16 = ml_dtypes.bfloat16

    def split(a):
        hi = a.astype(bf16)
        return hi, (a - hi.astype(f32)).astype(bf16)

    # bf16 weight panel [33, 466]
    wq33 = np.zeros((KC, 64), f32)
    wk33 = np.zeros((KC, 64), f32)
    selm = np.zeros((32, 64), f32)
    for h in range(H):
        wq33[0:32, 16 * h : 16 * h + 8] = w1[:, HD * h : HD * h + HD]
        wq33[32, 16 * h : 16 * h + 8] = b1[HD * h : HD * h + HD]
        wk33[0:32, 16 * h : 16 * h + 8] = w1[:, HID + HD * h : HID + HD * h + HD]
        wk33[32, 16 * h : 16 * h + 8] = b1[HID + HD * h : HID + HD * h + HD]
        for r in range(BOT):
            selm[BOT * h + r, 16 * h + 8 + r] = 1.0
    cpack2 = np.zeros((KC, 466), bf16)
    cpack2[:, 0:64], cpack2[:, 64:128] = split(wq33)
    cpack2[:, 128:192], cpack2[:, 192:256] = split(wk33)
    cpack2[:, 256:296], cpack2[:, 296:336] = split(wvv)
    cpack2[0:32, 336:400] = selm.astype(bf16)
    dwhi, dwlo = split(dw)
    cpack2[0:L, 400] = dwhi
    cpack2[0:L, 401] = dwlo
    gdwhi, gdwlo = split(gdw)
    cpack2[0, 402:434] = gdwhi
    cpack2[0, 434:466] = gdwlo

    ut = np.ascontiguousarray(u.transpose(0, 2, 1).reshape(H * BOT, N))
    vnn = np.ascontiguousarray(v.reshape(H * BOT, N))
    uthi, utlo = split(ut)
    vnhi, vnlo = split(vnn)

    common = {
        "cpack": cpack,
        "cpack2": cpack2,
        "uthi": uthi, "utlo": utlo, "vnhi": vnhi, "vnlo": vnlo,
    }

    xl = np.ascontiguousarray(x[:, T - 1 : T - 1 - L : -1, :])  # [B, L, N]
    xlhi = xl.astype(bf16)
    xllo = (xl - xlhi.astype(f32)).astype(bf16)

    in_maps = []
    for b in range(B):
        m = dict(common)
        m["xlhi"] = np.ascontiguousarray(xlhi[b])
        m["xllo"] = np.ascontiguousarray(xllo[b])
        m["feat"] = np.ascontiguousarray(features[b])
        m["featb"] = np.ascontiguousarray(features[b] + bo[None, :]).astype(f32)
        in_maps.append(m)

    loss = f32(np.exp(np.asarray(inputs["log_attn_reg"], f32)) / f32(N))
    return in_maps, loss


def kernel(**inputs):
    nc = _build_program()
    in_maps, loss = _host_prep(inputs)
    res = run_bass_kernel_spmd(nc, in_maps, core_ids=list(range(NCORES)))
    out = np.stack([res.results[b]["out"] for b in range(B)], axis=0)
    return out.astype(np.float32), loss
